# revision 1
# baseline (speedup 1.0000x reference)
"""ArteryMixer Trainium2 kernel: 8-core data-parallel over tokens.

Per-token math (B=2,S=2048,A=8,R=8,DIM=1024,H=8,HD=128,SC=16):
  qkv = concat(x+emb, res) @ Wqkv.T ; q,k rmsnorm ; k_res roped (folded into W);
  scores=elu(q@k.T/sqrt(HD)) ; mixed = scores@v/16 ; out = x + scale*(mixed@Wproj.T)

Device layout strategy (per core, 512 tokens):
  - All activations kept FEATURE-on-partitions (x.T etc., host pre-transposed).
  - QKV (q,k_art,k_res·Rope) via W-stationary GEMM -> qT/kT slabs (d-part, col=(t,slot)).
  - V via X-stationary GEMM -> v slabs in row layout (rows=(t,slot) on partitions).
  - artery-embed bias folded in as rank-8 extra matmul (one-hot trick).
  - rmsnorm: square (DVE) + gpsimd partition_all_reduce (f32 internal) + ACT ln/exp rsqrt,
    scale folded: rs_q = rsqrt(ssq/128+eps), rs_k = rsqrt(ssq+128*eps) (= rs*HD^-0.5).
  - attention per 16-token group: scoresT = kT_slice.T @ qT_slice (cross products),
    elu via Relu(ACT)+min(exp-1,0), block-diag mask*(1/16) kills cross-token terms.
  - mixedT = v.T @ routeT accumulated art+res -> feature-part layout feeds proj GEMM.
  - proj W-stationary -> projT ; y.T = projT*mixer_scale + x.T ; output stays transposed,
    host un-transposes.
"""

import numpy as np
import ml_dtypes

bf16 = ml_dtypes.bfloat16

HEADS = 8
HD = 128
DIM = 1024
MD = 1024
A = 8
RKV = 8
SC = 16
EPS = 1.1920929e-07
ROPE_BASE = 10000.0
N_CORES = 8
B, S = 2, 2048
TOK_PER_CORE = (B * S) // N_CORES  # 512
BLK_TOK = 64                        # tokens per pipeline block
NB = TOK_PER_CORE // BLK_TOK        # 8 blocks
CPB = BLK_TOK * 8                   # 512 cols per block (token-major, slot-minor)


def _rope_matrix():
    inv_freq = 1.0 / (ROPE_BASE ** (np.arange(0, HD, 2, dtype=np.float64) / HD))
    c, s = np.cos(inv_freq), np.sin(inv_freq)
    Rm = np.zeros((HD, HD), dtype=np.float64)
    i = np.arange(HD // 2)
    # reference _rope: out1 = x1*c + x2*s ; out2 = -x1*s + x2*c
    Rm[i, i] = c
    Rm[i, i + 64] = s
    Rm[i + 64, i] = -s
    Rm[i + 64, i + 64] = c
    return Rm


def build_program(tok_per_core=TOK_PER_CORE, repeat=1):
    import concourse.bass as bass  # noqa
    import concourse.mybir as mybir
    import concourse.tile as tile
    from concourse import bacc
    from concourse import bass_isa

    dt = mybir.dt
    Alu = mybir.AluOpType
    Act = mybir.ActivationFunctionType

    nb = tok_per_core // BLK_TOK
    COLS = tok_per_core * 8

    nc = bacc.Bacc(None, target_bir_lowering=False)

    xt_art = nc.dram_tensor("xt_art", [DIM, COLS], dt.bfloat16, kind="ExternalInput")
    xt_res = nc.dram_tensor("xt_res", [DIM, COLS], dt.bfloat16, kind="ExternalInput")
    wqkv_t = nc.dram_tensor("wqkv_t", [DIM, 3 * MD], dt.bfloat16, kind="ExternalInput")
    wv_t = nc.dram_tensor("wv_t", [DIM, MD], dt.bfloat16, kind="ExternalInput")
    wproj_t = nc.dram_tensor("wproj_t", [MD, DIM], dt.bfloat16, kind="ExternalInput")
    biasqk_d = nc.dram_tensor("biasqk", [128, 128], dt.bfloat16, kind="ExternalInput")
    biasv_d = nc.dram_tensor("biasv", [128, MD], dt.bfloat16, kind="ExternalInput")
    mask_d = nc.dram_tensor("mask", [128, 128], dt.bfloat16, kind="ExternalInput")
    mscale_d = nc.dram_tensor("mscale", [128, 8], dt.float32, kind="ExternalInput")
    out_t = nc.dram_tensor("out_t", [DIM, COLS], dt.bfloat16, kind="ExternalOutput")

    with tile.TileContext(nc) as tc:
        with (
            tc.tile_pool(name="w", bufs=1) as wpool,
            tc.tile_pool(name="x", bufs=2) as xpool,
            tc.tile_pool(name="slab", bufs=2) as spool,
            tc.tile_pool(name="vslab", bufs=1) as vpool,
            tc.tile_pool(name="nrm", bufs=2) as npool,
            tc.tile_pool(name="att", bufs=2) as fpool,
            tc.tile_pool(name="rtp", bufs=3) as rtpool,
            tc.tile_pool(name="y", bufs=2) as ypool,
            tc.tile_pool(name="mm", bufs=2, space="PSUM") as mmpool,
            tc.tile_pool(name="sc", bufs=2, space="PSUM") as scpool,
            tc.tile_pool(name="mx", bufs=1, space="PSUM") as mxpool,
        ):
            # ---- resident weights/constants ----
            wqkv_sb = wpool.tile([128, 8, 3 * MD], dt.bfloat16)
            nc.sync.dma_start(
                wqkv_sb, wqkv_t[:].rearrange("(dc p) f -> p dc f", p=128)
            )
            wv_sb = wpool.tile([128, 8, MD], dt.bfloat16)
            nc.sync.dma_start(wv_sb, wv_t[:].rearrange("(dc p) f -> p dc f", p=128))
            wproj_sb = wpool.tile([128, 8, DIM], dt.bfloat16)
            nc.sync.dma_start(
                wproj_sb, wproj_t[:].rearrange("(mc p) f -> p mc f", p=128)
            )
            biasqk_sb = wpool.tile([128, 16, 8], dt.bfloat16)
            nc.sync.dma_start(biasqk_sb, biasqk_d[:].rearrange("p (fc a) -> p fc a", a=8))
            biasv_sb = wpool.tile([128, MD], dt.bfloat16)
            nc.sync.dma_start(biasv_sb, biasv_d[:])
            mask_sb = wpool.tile([128, 128], dt.bfloat16)
            nc.sync.dma_start(mask_sb, mask_d[:])
            mscale_sb = wpool.tile([128, 8], dt.float32)
            nc.sync.dma_start(mscale_sb, mscale_d[:])
            eps_q = wpool.tile([128, 1], dt.float32)
            nc.vector.memset(eps_q, EPS)
            eps_k = wpool.tile([128, 1], dt.float32)
            nc.vector.memset(eps_k, HD * EPS)

            xa_dram = xt_art[:].rearrange("(dc p) c -> p dc c", p=128)
            xr_dram = xt_res[:].rearrange("(dc p) c -> p dc c", p=128)
            yo_dram = out_t[:].rearrange("(dc p) c -> p dc c", p=128)

            def build_gemm_items(blk):
                """Allocate block tiles + return GEMM work-item closures."""
                c0 = blk * CPB
                xa_h = [xpool.tile([128, 4, CPB], dt.bfloat16, tag=f"xa{i}", name=f"xa{i}")
                        for i in range(2)]
                xr_h = [xpool.tile([128, 4, CPB], dt.bfloat16, tag=f"xr{i}", name=f"xr{i}")
                        for i in range(2)]
                for i in range(2):
                    nc.sync.dma_start(
                        xa_h[i], xa_dram[:, i * 4 : i * 4 + 4, c0 : c0 + CPB]
                    )
                    nc.sync.dma_start(
                        xr_h[i], xr_dram[:, i * 4 : i * 4 + 4, c0 : c0 + CPB]
                    )
                qT = spool.tile([128, 8, CPB], dt.bfloat16, tag="qT")
                kTa = spool.tile([128, 8, CPB], dt.bfloat16, tag="kTa")
                kTr = spool.tile([128, 8, CPB], dt.bfloat16, tag="kTr")
                va = vpool.tile([128, 4, 8, HD], dt.bfloat16, tag="va")
                vr = vpool.tile([128, 4, 8, HD], dt.bfloat16, tag="vr")
                st = dict(xa_h=xa_h, xr_h=xr_h, qT=qT, kTa=kTa, kTr=kTr,
                          va=va, vr=vr, c0=c0)
                slabs = [qT, kTa, kTr]
                items = []

                def qkv_item(fc):
                    def go():
                        ps = mmpool.tile([128, CPB], dt.float32, tag="mmps")
                        halves = xr_h if fc >= 16 else xa_h
                        for dc in range(8):
                            nc.tensor.matmul(
                                ps,
                                wqkv_sb[:, dc, fc * 128 : (fc + 1) * 128],
                                halves[dc // 4][:, dc % 4, :],
                                start=(dc == 0),
                                stop=(dc == 7),
                            )
                        dst = slabs[fc // 8][:, fc % 8, :]
                        nc.scalar.copy(out=dst, in_=ps)
                        if fc < 16:
                            d3 = dst.rearrange("p (t a) -> p t a", a=8)
                            nc.vector.tensor_add(
                                d3,
                                d3,
                                biasqk_sb[:, fc, None, :].to_broadcast(
                                    (128, BLK_TOK, 8)
                                ),
                            )
                    return go

                def v_item(isart, rc, vh):
                    def go():
                        halves, dstv = (xa_h, va) if isart else (xr_h, vr)
                        ps = mmpool.tile([128, 512], dt.float32, tag="mmps")
                        for dc in range(8):
                            nc.tensor.matmul(
                                ps,
                                halves[dc // 4][:, dc % 4, rc * 128 : (rc + 1) * 128],
                                wv_sb[:, dc, vh * 512 : (vh + 1) * 512],
                                start=(dc == 0),
                                stop=(dc == 7),
                            )
                        dv = dstv[:, rc, vh * 4 : (vh + 1) * 4, :]
                        nc.scalar.copy(out=dv, in_=ps)
                        if isart:
                            nc.vector.tensor_add(
                                dv, dv, biasv_sb[:, vh * 512 : (vh + 1) * 512]
                            )
                    return go

                for fc in range(24):
                    items.append(qkv_item(fc))
                for isart in (True, False):
                    for rc in range(4):
                        for vh in range(2):
                            items.append(v_item(isart, rc, vh))
                return st, items

            def build_attn_items(st):
                """Work items for norm + attention + proj of a block."""
                qT, kTa, kTr = st["qT"], st["kTa"], st["kTr"]
                va, vr, xa_h, c0 = st["va"], st["vr"], st["xa_h"], st["c0"]
                items = []

                def norm_item(slab, epsv, scv, hh):
                    def go():
                        with nc.allow_low_precision(
                            reason="all-reduce upcasts internally; bf16 ~0.4%"
                        ):
                            sl = slab[:, hh, :]
                            sq = npool.tile([128, CPB], dt.bfloat16, tag="nsq")
                            nc.vector.tensor_mul(sq, sl, sl)
                            ssq = npool.tile([128, CPB], dt.bfloat16, tag="nssq")
                            nc.gpsimd.partition_all_reduce(
                                ssq, sq, channels=128,
                                reduce_op=bass_isa.ReduceOp.add,
                            )
                            rt = npool.tile([128, CPB], dt.bfloat16, tag="nsq")
                            nc.scalar.activation(
                                rt, ssq, Act.Sqrt, bias=epsv, scale=scv
                            )
                            rs = npool.tile([128, CPB], dt.bfloat16, tag="nssq")
                            nc.vector.reciprocal(rs, rt)
                            nc.vector.tensor_mul(sl, sl, rs)
                    return go

                mixedT = vpool.tile([128, 8, CPB], dt.bfloat16, tag="mixedT")
                st["mixedT"] = mixedT
                routes = {}

                def scores_item(g, half):
                    def go():
                        gsl = slice(g * 128, (g + 1) * 128)
                        kT = kTa if half == 0 else kTr
                        ps = scpool.tile([128, 8, 128], dt.float32, tag="scps")
                        for h in range(8):
                            nc.tensor.matmul(
                                ps[:, h, :], kT[:, h, gsl], qT[:, h, gsl],
                                start=True, stop=True,
                            )
                        esc = fpool.tile([128, 8, 128], dt.bfloat16, tag="ers")
                        rsc = fpool.tile([128, 8, 128], dt.bfloat16, tag="ers")
                        nc.scalar.activation(esc, ps, Act.Exp)
                        nc.scalar.activation(rsc, ps, Act.Relu)
                        # elu = relu(s) + (min(exp(s),1) - 1)
                        nc.vector.tensor_scalar(esc, esc, 1.0, -1.0, Alu.min, Alu.add)
                        nc.vector.tensor_add(esc, rsc, esc)
                        route = rtpool.tile([128, 8, 128], dt.bfloat16, tag="rt")
                        nc.vector.tensor_mul(
                            route, esc,
                            mask_sb[:, None, :].to_broadcast((128, 8, 128)),
                        )
                        routes[(g, half)] = route
                    return go

                def mixed_item(g):
                    def go():
                        gsl = slice(g * 128, (g + 1) * 128)
                        mx = mxpool.tile([128, 8, 128], dt.float32, tag="mxps")
                        for h in range(8):
                            nc.tensor.matmul(
                                mx[:, h, :], va[:, g, h, :],
                                routes[(g, 0)][:, h, :], start=True, stop=False,
                            )
                            nc.tensor.matmul(
                                mx[:, h, :], vr[:, g, h, :],
                                routes[(g, 1)][:, h, :], start=False, stop=True,
                            )
                        nc.scalar.copy(out=mixedT[:, :, gsl], in_=mx)
                    return go

                def proj_item(dc):
                    def go():
                        ps = mmpool.tile([128, CPB], dt.float32, tag="mmps")
                        for h in range(8):
                            nc.tensor.matmul(
                                ps,
                                wproj_sb[:, h, dc * 128 : (dc + 1) * 128],
                                mixedT[:, h, :],
                                start=(h == 0), stop=(h == 7),
                            )
                        yb = ypool.tile([128, CPB], dt.bfloat16, tag="yb")
                        nc.vector.scalar_tensor_tensor(
                            out=yb, in0=ps, scalar=mscale_sb[:, dc : dc + 1],
                            in1=xa_h[dc // 4][:, dc % 4, :],
                            op0=Alu.mult, op1=Alu.add,
                        )
                        nc.sync.dma_start(yo_dram[:, dc, c0 : c0 + CPB], yb)
                    return go

                norm_items = []
                for slab, epsv, scv in (
                    (qT, eps_q, 1.0 / HD),
                    (kTa, eps_k, 1.0),
                    (kTr, eps_k, 1.0),
                ):
                    for hh in range(8):
                        norm_items.append(norm_item(slab, epsv, scv, hh))
                return dict(
                    norm=norm_items,
                    groups=[(scores_item(g, 0), scores_item(g, 1), mixed_item(g))
                            for g in range(4)],
                    proj=[proj_item(dc) for dc in range(8)],
                )

            def merge(attn, gemm):
                """Structured interleave: norm 3:1 with gemms, then per group
                sc,G,sc,G,G,mx,G, then proj 1:2 with gemms; leftovers last."""
                out = []
                gq = list(gemm)

                def g(n):
                    for _ in range(n):
                        if gq:
                            out.append(gq.pop(0))

                if attn is None:
                    return list(gemm)
                for i, it in enumerate(attn["norm"]):
                    out.append(it)
                    if i % 3 == 2:
                        g(1)
                for sc0, sc1, mx in attn["groups"]:
                    out.append(sc0); g(1)
                    out.append(sc1); g(2)
                    out.append(mx); g(1)
                for p in attn["proj"]:
                    out.append(p); g(2)
                out.extend(gq)
                return out

            blklist = [b for _ in range(repeat) for b in range(nb)]
            prev_st = None
            for i in range(len(blklist) + 1):
                gemm_items = []
                if i < len(blklist):
                    st, gemm_items = build_gemm_items(blklist[i])
                attn = build_attn_items(prev_st) if prev_st is not None else None
                for item in merge(attn, gemm_items):
                    item()
                if i < len(blklist):
                    prev_st = st

    nc.compile()
    return nc


def host_prep(x, artery_embed, residual_kv, Wqkv, Wproj, mixer_scale,
              tok_per_core=TOK_PER_CORE, n_cores=N_CORES):
    T = x.shape[0] * x.shape[1]
    x_flat = np.asarray(x, dtype=np.float32).reshape(T, A, DIM)
    res_flat = np.asarray(residual_kv, dtype=np.float32).reshape(T, RKV, DIM)

    Rm = _rope_matrix()
    Wq = np.asarray(Wqkv[0:MD], dtype=np.float64)
    Wk = np.asarray(Wqkv[MD : 2 * MD], dtype=np.float64)
    Wv = np.asarray(Wqkv[2 * MD : 3 * MD], dtype=np.float64)
    Wk_res = np.einsum("de,hec->hdc", Rm, Wk.reshape(HEADS, HD, DIM)).reshape(MD, DIM)

    wqkv_t = np.ascontiguousarray(
        np.concatenate([Wq, Wk, Wk_res], axis=0).T
    ).astype(bf16)
    wv_t = np.ascontiguousarray(Wv.T).astype(bf16)
    wproj_t = np.ascontiguousarray(np.asarray(Wproj, dtype=np.float64).T).astype(bf16)

    emb = np.asarray(artery_embed, dtype=np.float64)
    bias_q = emb @ Wq.T
    bias_k = emb @ Wk.T
    bias_v = emb @ Wv.T
    # biasqk[p, fc*8+a] = bias_cat[a, fc*128+p]
    bias_cat = np.concatenate([bias_q, bias_k], axis=1)  # (8, 2048)
    biasqk = np.ascontiguousarray(
        bias_cat.reshape(8, 16, 128).transpose(2, 1, 0).reshape(128, 128)
    ).astype(bf16)
    biasv = np.ascontiguousarray(np.tile(bias_v, (16, 1))).astype(bf16)

    mask = np.zeros((128, 128), dtype=np.float32)
    for t in range(16):
        mask[t * 8 : (t + 1) * 8, t * 8 : (t + 1) * 8] = 1.0 / SC
    mask = mask.astype(bf16)

    mscale = np.ascontiguousarray(
        np.asarray(mixer_scale, dtype=np.float32).reshape(8, 128).T
    )

    shared = dict(
        wqkv_t=wqkv_t, wv_t=wv_t, wproj_t=wproj_t, biasqk=biasqk, biasv=biasv,
        mask=mask, mscale=mscale,
    )
    in_maps = []
    for i in range(n_cores):
        sl = slice(i * tok_per_core, (i + 1) * tok_per_core)
        xa = np.ascontiguousarray(
            x_flat[sl].reshape(tok_per_core * A, DIM).T
        ).astype(bf16)
        xr = np.ascontiguousarray(
            res_flat[sl].reshape(tok_per_core * RKV, DIM).T
        ).astype(bf16)
        m = dict(shared)
        m["xt_art"] = xa
        m["xt_res"] = xr
        in_maps.append(m)
    return in_maps


def assemble_output(outs, tok_per_core=TOK_PER_CORE):
    """outs: list of (DIM, tok_per_core*8) bf16 arrays -> (B,S,A,DIM) f32."""
    parts = []
    for o in outs:
        y = np.asarray(o, dtype=np.float32)  # (1024, T*8)
        parts.append(y.reshape(DIM, tok_per_core, A).transpose(1, 2, 0))
    full = np.concatenate(parts, axis=0)  # (n_tok, A, DIM)
    if full.shape[0] == B * S:
        full = full.reshape(B, S, A, DIM)
    return np.ascontiguousarray(full)


_NC_CACHE = {}


def kernel(x, artery_embed, residual_kv, Wqkv, Wproj, mixer_scale):
    from concourse.bass_utils import run_bass_kernel_spmd

    key = TOK_PER_CORE
    if key not in _NC_CACHE:
        _NC_CACHE[key] = build_program(TOK_PER_CORE)
    nc = _NC_CACHE[key]

    in_maps = host_prep(x, artery_embed, residual_kv, Wqkv, Wproj, mixer_scale)
    res = run_bass_kernel_spmd(nc, in_maps, core_ids=list(range(N_CORES)))
    outs = [r["out_t"] for r in res.results]
    return assemble_output(outs)



# revision 15
# speedup vs baseline: 61.2010x; 61.2010x over previous
"""ArteryMixer Trainium2 kernel: 8-core data-parallel over tokens.

Per-token math (B=2,S=2048,A=8,R=8,DIM=1024,H=8,HD=128,SC=16):
  qkv = concat(x+emb, res) @ Wqkv.T ; q,k rmsnorm ; k_res roped (folded into W);
  scores=elu(q@k.T/sqrt(HD)) ; mixed = scores@v/16 ; out = x + scale*(mixed@Wproj.T)

Device layout strategy (per core, 512 tokens):
  - All activations kept FEATURE-on-partitions (x.T etc., host pre-transposed).
  - QKV (q,k_art,k_res·Rope) via W-stationary GEMM -> qT/kT slabs (d-part, col=(t,slot)).
  - V via X-stationary GEMM -> v slabs in row layout (rows=(t,slot) on partitions).
  - artery-embed bias folded in as rank-8 extra matmul (one-hot trick).
  - rmsnorm: square (DVE) + gpsimd partition_all_reduce (f32 internal) + ACT ln/exp rsqrt,
    scale folded: rs_q = rsqrt(ssq/128+eps), rs_k = rsqrt(ssq+128*eps) (= rs*HD^-0.5).
  - attention per 16-token group: scoresT = kT_slice.T @ qT_slice (cross products),
    elu via Relu(ACT)+min(exp-1,0), block-diag mask*(1/16) kills cross-token terms.
  - mixedT = v.T @ routeT accumulated art+res -> feature-part layout feeds proj GEMM.
  - proj W-stationary -> projT ; y.T = projT*mixer_scale + x.T ; output stays transposed,
    host un-transposes.
"""

import numpy as np
import ml_dtypes

bf16 = ml_dtypes.bfloat16

HEADS = 8
HD = 128
DIM = 1024
MD = 1024
A = 8
RKV = 8
SC = 16
EPS = 1.1920929e-07
ROPE_BASE = 10000.0
N_CORES = 8
B, S = 2, 2048
TOK_PER_CORE = (B * S) // N_CORES  # 512
BLK_TOK = 64                        # tokens per pipeline block
NB = TOK_PER_CORE // BLK_TOK        # 8 blocks
CPB = BLK_TOK * 8                   # 512 cols per block (token-major, slot-minor)


def _rope_matrix():
    inv_freq = 1.0 / (ROPE_BASE ** (np.arange(0, HD, 2, dtype=np.float64) / HD))
    c, s = np.cos(inv_freq), np.sin(inv_freq)
    Rm = np.zeros((HD, HD), dtype=np.float64)
    i = np.arange(HD // 2)
    # reference _rope: out1 = x1*c + x2*s ; out2 = -x1*s + x2*c
    Rm[i, i] = c
    Rm[i, i + 64] = s
    Rm[i + 64, i] = -s
    Rm[i + 64, i + 64] = c
    return Rm


def build_program(tok_per_core=TOK_PER_CORE, repeat=1):
    import concourse.bass as bass  # noqa
    import concourse.mybir as mybir
    import concourse.tile as tile
    from concourse import bacc
    from concourse import bass_isa

    dt = mybir.dt
    Alu = mybir.AluOpType
    Act = mybir.ActivationFunctionType

    nb = tok_per_core // BLK_TOK
    COLS = tok_per_core * 8

    nc = bacc.Bacc(None, target_bir_lowering=False)

    xt_art = nc.dram_tensor("xt_art", [DIM, COLS], dt.bfloat16, kind="ExternalInput")
    xt_res = nc.dram_tensor("xt_res", [DIM, COLS], dt.bfloat16, kind="ExternalInput")
    wqkv_t = nc.dram_tensor("wqkv_t", [DIM, 3 * MD], dt.bfloat16, kind="ExternalInput")
    wv_t = nc.dram_tensor("wv_t", [DIM, MD], dt.bfloat16, kind="ExternalInput")
    wproj_t = nc.dram_tensor("wproj_t", [MD, DIM], dt.bfloat16, kind="ExternalInput")
    biasqk_d = nc.dram_tensor("biasqk", [128, 128], dt.bfloat16, kind="ExternalInput")
    biasv_d = nc.dram_tensor("biasv", [128, MD], dt.bfloat16, kind="ExternalInput")
    mask_d = nc.dram_tensor("mask", [128, 128], dt.bfloat16, kind="ExternalInput")
    mscale_d = nc.dram_tensor("mscale", [128, 8], dt.float32, kind="ExternalInput")
    out_t = nc.dram_tensor("out_t", [DIM, COLS], dt.bfloat16, kind="ExternalOutput")

    with tile.TileContext(nc) as tc:
        with (
            tc.tile_pool(name="w", bufs=1) as wpool,
            tc.tile_pool(name="x", bufs=2) as xpool,
            tc.tile_pool(name="slab", bufs=2) as spool,
            tc.tile_pool(name="vslab", bufs=1) as vpool,
            tc.tile_pool(name="nrm", bufs=2) as npool,
            tc.tile_pool(name="att", bufs=2) as fpool,
            tc.tile_pool(name="rtp", bufs=3) as rtpool,
            tc.tile_pool(name="y", bufs=2) as ypool,
            tc.tile_pool(name="mm", bufs=2, space="PSUM") as mmpool,
            tc.tile_pool(name="sc", bufs=1, space="PSUM") as scpool,
            tc.tile_pool(name="mx", bufs=1, space="PSUM") as mxpool,
            tc.tile_pool(name="sq", bufs=2, space="PSUM") as sqpool,
        ):
            # ---- resident weights/constants ----
            wqkv_sb = wpool.tile([128, 8, 3 * MD], dt.bfloat16)
            nc.sync.dma_start(
                wqkv_sb, wqkv_t[:].rearrange("(dc p) f -> p dc f", p=128)
            )
            wv_sb = wpool.tile([128, 8, MD], dt.bfloat16)
            nc.sync.dma_start(wv_sb, wv_t[:].rearrange("(dc p) f -> p dc f", p=128))
            wproj_sb = wpool.tile([128, 8, DIM], dt.bfloat16)
            nc.sync.dma_start(
                wproj_sb, wproj_t[:].rearrange("(mc p) f -> p mc f", p=128)
            )
            biasqk_sb = wpool.tile([128, 16, 8], dt.bfloat16)
            nc.sync.dma_start(biasqk_sb, biasqk_d[:].rearrange("p (fc a) -> p fc a", a=8))
            biasv_sb = wpool.tile([128, MD], dt.bfloat16)
            nc.sync.dma_start(biasv_sb, biasv_d[:])
            mask_sb = wpool.tile([128, 128], dt.bfloat16)
            nc.sync.dma_start(mask_sb, mask_d[:])
            mscale_sb = wpool.tile([128, 8], dt.float32)
            nc.sync.dma_start(mscale_sb, mscale_d[:])
            eps_q = wpool.tile([128, 1], dt.float32)
            nc.vector.memset(eps_q, EPS)
            eps_k = wpool.tile([128, 1], dt.float32)
            nc.vector.memset(eps_k, HD * EPS)
            ones_sb = wpool.tile([128, 128], dt.bfloat16)
            nc.vector.memset(ones_sb, 1.0)

            xa_dram = xt_art[:].rearrange("(dc p) c -> p dc c", p=128)
            xr_dram = xt_res[:].rearrange("(dc p) c -> p dc c", p=128)
            yo_dram = out_t[:].rearrange("(dc p) c -> p dc c", p=128)

            def build_gemm_items(blk):
                """Allocate block tiles + return GEMM work-item closures."""
                c0 = blk * CPB
                xa_h = [xpool.tile([128, 4, CPB], dt.bfloat16, tag=f"xa{i}", name=f"xa{i}")
                        for i in range(2)]
                xr_h = [xpool.tile([128, 4, CPB], dt.bfloat16, tag=f"xr{i}", name=f"xr{i}")
                        for i in range(2)]
                for i in range(2):
                    nc.sync.dma_start(
                        xa_h[i], xa_dram[:, i * 4 : i * 4 + 4, c0 : c0 + CPB]
                    )
                    nc.sync.dma_start(
                        xr_h[i], xr_dram[:, i * 4 : i * 4 + 4, c0 : c0 + CPB]
                    )
                qT = spool.tile([128, 8, CPB], dt.bfloat16, tag="qT")
                kTa = spool.tile([128, 8, CPB], dt.bfloat16, tag="kTa")
                kTr = spool.tile([128, 8, CPB], dt.bfloat16, tag="kTr")
                va = vpool.tile([128, 4, 8, HD], dt.bfloat16, tag="va")
                vr = vpool.tile([128, 4, 8, HD], dt.bfloat16, tag="vr")
                st = dict(xa_h=xa_h, xr_h=xr_h, qT=qT, kTa=kTa, kTr=kTr,
                          va=va, vr=vr, c0=c0)
                slabs = [qT, kTa, kTr]
                items = []

                def qkv_item(fc):
                    def go():
                        ps = mmpool.tile([128, CPB], dt.float32, tag="mmps")
                        halves = xr_h if fc >= 16 else xa_h
                        for dc in range(8):
                            nc.tensor.matmul(
                                ps,
                                wqkv_sb[:, dc, fc * 128 : (fc + 1) * 128],
                                halves[dc // 4][:, dc % 4, :],
                                start=(dc == 0),
                                stop=(dc == 7),
                            )
                        dst = slabs[fc // 8][:, fc % 8, :]
                        nc.scalar.copy(out=dst, in_=ps)
                        if fc < 16:
                            d3 = dst.rearrange("p (t a) -> p t a", a=8)
                            nc.vector.tensor_add(
                                d3,
                                d3,
                                biasqk_sb[:, fc, None, :].to_broadcast(
                                    (128, BLK_TOK, 8)
                                ),
                            )
                    return go

                def v_item(isart, rc, vh):
                    def go():
                        halves, dstv = (xa_h, va) if isart else (xr_h, vr)
                        ps = mmpool.tile([128, 512], dt.float32, tag="mmps")
                        for dc in range(8):
                            nc.tensor.matmul(
                                ps,
                                halves[dc // 4][:, dc % 4, rc * 128 : (rc + 1) * 128],
                                wv_sb[:, dc, vh * 512 : (vh + 1) * 512],
                                start=(dc == 0),
                                stop=(dc == 7),
                            )
                        dv = dstv[:, rc, vh * 4 : (vh + 1) * 4, :]
                        nc.scalar.copy(out=dv, in_=ps)
                        if isart:
                            nc.vector.tensor_add(
                                dv, dv, biasv_sb[:, vh * 512 : (vh + 1) * 512]
                            )
                    return go

                for fc in range(24):
                    items.append(qkv_item(fc))
                for isart in (True, False):
                    for rc in range(4):
                        for vh in range(2):
                            items.append(v_item(isart, rc, vh))
                return st, items

            def build_attn_items(st):
                """Work items for norm + attention + proj of a block."""
                qT, kTa, kTr = st["qT"], st["kTa"], st["kTr"]
                va, vr, xa_h, c0 = st["va"], st["vr"], st["xa_h"], st["c0"]
                items = []

                def norm_items_for(slab, epsv, scv):
                    """rmsnorm a slab: ssq per head via ones-matmul (partition
                    reduce on PE) into rows {0,32,64} of a shared psum bank,
                    batched sqrt+recip on the strided rows, then per-head
                    rank-1 broadcast matmul + DVE multiply."""
                    groups = [(0, [0, 1, 2]), (1, [3, 4, 5]), (2, [6, 7])]
                    rs_tiles = {}
                    for gi, _ in groups:
                        rs_tiles[gi] = npool.tile(
                            [128, CPB], dt.bfloat16, tag="nrs", name=f"nrs{gi}")

                    def stat_item(gi, heads):
                        def go():
                            with nc.allow_low_precision(reason="bf16 norm ~0.4%"):
                                ssq = sqpool.tile([128, CPB], dt.float32, tag="ssq")
                                np_ = 32 * (len(heads) - 1) + 1
                                nc.vector.memset(ssq[0:np_, :], 1.0)
                                for j, hh in enumerate(heads):
                                    sq = npool.tile(
                                        [128, CPB], dt.bfloat16, tag="nsq")
                                    nc.vector.tensor_mul(
                                        sq, slab[:, hh, :], slab[:, hh, :])
                                    nc.tensor.matmul(
                                        ssq[32 * j : 32 * j + 1, :],
                                        ones_sb[:, 0:1], sq,
                                        start=True, stop=True,
                                    )
                                # one batched sqrt+recip over the contiguous
                                # partition range; rows between 32j anchors
                                # hold memset filler and are never read
                                rt = npool.tile([128, CPB], dt.bfloat16, tag="nrt")
                                nc.scalar.activation(
                                    rt[0:np_, :], ssq[0:np_, :], Act.Sqrt,
                                    bias=epsv[0:np_, :], scale=scv,
                                )
                                nc.vector.reciprocal(
                                    rs_tiles[gi][0:np_, :], rt[0:np_, :])
                        return go

                    def apply_item(gi, j_heads):
                        def go():
                            rs = rs_tiles[gi]
                            for j, hh in j_heads:
                                bc = mmpool.tile([128, CPB], dt.float32, tag="mmps")
                                nc.tensor.matmul(
                                    bc,
                                    ones_sb[32 * j : 32 * j + 1, :],
                                    rs[32 * j : 32 * j + 1, :],
                                    start=True, stop=True,
                                )
                                nc.vector.tensor_mul(
                                    slab[:, hh, :], slab[:, hh, :], bc
                                )
                        return go

                    stats = [stat_item(gi, heads) for gi, heads in groups]
                    applies = []
                    for gi, heads in groups:
                        pairs = list(enumerate(heads))
                        applies.append(apply_item(gi, pairs[:2]))
                        if len(pairs) > 2:
                            applies.append(apply_item(gi, pairs[2:]))
                    return stats, applies

                mixedT = vpool.tile([128, 8, CPB], dt.bfloat16, tag="mixedT")
                st["mixedT"] = mixedT
                routes = {}

                def scores_item(g, half):
                    def go():
                        gsl = slice(g * 128, (g + 1) * 128)
                        kT = kTa if half == 0 else kTr
                        ps = scpool.tile([128, 8, 128], dt.float32, tag="scps")
                        for h in range(8):
                            nc.tensor.matmul(
                                ps[:, h, :], kT[:, h, gsl], qT[:, h, gsl],
                                start=True, stop=True,
                            )
                        esc = fpool.tile([128, 8, 128], dt.bfloat16, tag="ers")
                        rsc = fpool.tile([128, 8, 128], dt.bfloat16, tag="ers")
                        nc.scalar.activation(esc, ps, Act.Exp)
                        nc.scalar.activation(rsc, ps, Act.Relu)
                        # elu = relu(s) + (min(exp(s),1) - 1)
                        nc.vector.tensor_scalar(esc, esc, 1.0, -1.0, Alu.min, Alu.add)
                        nc.vector.tensor_add(esc, rsc, esc)
                        route = rtpool.tile([128, 8, 128], dt.bfloat16, tag="rt")
                        nc.vector.tensor_mul(
                            route, esc,
                            mask_sb[:, None, :].to_broadcast((128, 8, 128)),
                        )
                        routes[(g, half)] = route
                    return go

                def mixed_item(g):
                    def go():
                        gsl = slice(g * 128, (g + 1) * 128)
                        mx = mxpool.tile([128, 8, 128], dt.float32, tag="mxps")
                        for h in range(8):
                            nc.tensor.matmul(
                                mx[:, h, :], va[:, g, h, :],
                                routes[(g, 0)][:, h, :], start=True, stop=False,
                            )
                            nc.tensor.matmul(
                                mx[:, h, :], vr[:, g, h, :],
                                routes[(g, 1)][:, h, :], start=False, stop=True,
                            )
                        nc.scalar.copy(out=mixedT[:, :, gsl], in_=mx)
                    return go

                def proj_item(dc):
                    def go():
                        ps = mmpool.tile([128, CPB], dt.float32, tag="mmps")
                        for h in range(8):
                            nc.tensor.matmul(
                                ps,
                                wproj_sb[:, h, dc * 128 : (dc + 1) * 128],
                                mixedT[:, h, :],
                                start=(h == 0), stop=(h == 7),
                            )
                        yb = ypool.tile([128, CPB], dt.bfloat16, tag="yb")
                        nc.vector.scalar_tensor_tensor(
                            out=yb, in0=ps, scalar=mscale_sb[:, dc : dc + 1],
                            in1=xa_h[dc // 4][:, dc % 4, :],
                            op0=Alu.mult, op1=Alu.add,
                        )
                        nc.sync.dma_start(yo_dram[:, dc, c0 : c0 + CPB], yb)
                    return go

                all_stats, all_applies = [], []
                for slab, epsv, scv in (
                    (qT, eps_q, 1.0 / HD),
                    (kTa, eps_k, 1.0),
                    (kTr, eps_k, 1.0),
                ):
                    stats, applies = norm_items_for(slab, epsv, scv)
                    all_stats.extend(stats)
                    all_applies.extend(applies)
                norm_items = all_stats + all_applies
                return dict(
                    norm=norm_items,
                    groups=[(scores_item(g, 0), scores_item(g, 1), mixed_item(g))
                            for g in range(4)],
                    proj=[proj_item(dc) for dc in range(8)],
                )

            def merge(attn, gemm):
                """Structured interleave: norm 3:1 with gemms, then per group
                sc,G,sc,G,G,mx,G, then proj 1:2 with gemms; leftovers last."""
                out = []
                gq = list(gemm)

                def g(n):
                    for _ in range(n):
                        if gq:
                            out.append(gq.pop(0))

                if attn is None:
                    return list(gemm)
                for i, it in enumerate(attn["norm"]):
                    out.append(it)
                    if i % 3 == 2:
                        g(1)
                for sc0, sc1, mx in attn["groups"]:
                    out.append(sc0); g(1)
                    out.append(sc1); g(2)
                    out.append(mx); g(1)
                for p in attn["proj"]:
                    out.append(p); g(2)
                out.extend(gq)
                return out

            blklist = [b for _ in range(repeat) for b in range(nb)]
            prev_st = None
            for i in range(len(blklist) + 1):
                gemm_items = []
                if i < len(blklist):
                    st, gemm_items = build_gemm_items(blklist[i])
                attn = build_attn_items(prev_st) if prev_st is not None else None
                for item in merge(attn, gemm_items):
                    item()
                if i < len(blklist):
                    prev_st = st

    nc.compile()
    return nc


def host_prep(x, artery_embed, residual_kv, Wqkv, Wproj, mixer_scale,
              tok_per_core=TOK_PER_CORE, n_cores=N_CORES):
    T = x.shape[0] * x.shape[1]
    x_flat = np.asarray(x, dtype=np.float32).reshape(T, A, DIM)
    res_flat = np.asarray(residual_kv, dtype=np.float32).reshape(T, RKV, DIM)

    Rm = _rope_matrix()
    Wq = np.asarray(Wqkv[0:MD], dtype=np.float64)
    Wk = np.asarray(Wqkv[MD : 2 * MD], dtype=np.float64)
    Wv = np.asarray(Wqkv[2 * MD : 3 * MD], dtype=np.float64)
    Wk_res = np.einsum("de,hec->hdc", Rm, Wk.reshape(HEADS, HD, DIM)).reshape(MD, DIM)

    wqkv_t = np.ascontiguousarray(
        np.concatenate([Wq, Wk, Wk_res], axis=0).T
    ).astype(bf16)
    wv_t = np.ascontiguousarray(Wv.T).astype(bf16)
    wproj_t = np.ascontiguousarray(np.asarray(Wproj, dtype=np.float64).T).astype(bf16)

    emb = np.asarray(artery_embed, dtype=np.float64)
    bias_q = emb @ Wq.T
    bias_k = emb @ Wk.T
    bias_v = emb @ Wv.T
    # biasqk[p, fc*8+a] = bias_cat[a, fc*128+p]
    bias_cat = np.concatenate([bias_q, bias_k], axis=1)  # (8, 2048)
    biasqk = np.ascontiguousarray(
        bias_cat.reshape(8, 16, 128).transpose(2, 1, 0).reshape(128, 128)
    ).astype(bf16)
    biasv = np.ascontiguousarray(np.tile(bias_v, (16, 1))).astype(bf16)

    mask = np.zeros((128, 128), dtype=np.float32)
    for t in range(16):
        mask[t * 8 : (t + 1) * 8, t * 8 : (t + 1) * 8] = 1.0 / SC
    mask = mask.astype(bf16)

    mscale = np.ascontiguousarray(
        np.asarray(mixer_scale, dtype=np.float32).reshape(8, 128).T
    )

    shared = dict(
        wqkv_t=wqkv_t, wv_t=wv_t, wproj_t=wproj_t, biasqk=biasqk, biasv=biasv,
        mask=mask, mscale=mscale,
    )
    in_maps = []
    for i in range(n_cores):
        sl = slice(i * tok_per_core, (i + 1) * tok_per_core)
        xa = np.ascontiguousarray(
            x_flat[sl].reshape(tok_per_core * A, DIM).T
        ).astype(bf16)
        xr = np.ascontiguousarray(
            res_flat[sl].reshape(tok_per_core * RKV, DIM).T
        ).astype(bf16)
        m = dict(shared)
        m["xt_art"] = xa
        m["xt_res"] = xr
        in_maps.append(m)
    return in_maps


def assemble_output(outs, tok_per_core=TOK_PER_CORE):
    """outs: list of (DIM, tok_per_core*8) bf16 arrays -> (B,S,A,DIM) f32."""
    parts = []
    for o in outs:
        y = np.asarray(o, dtype=np.float32)  # (1024, T*8)
        parts.append(y.reshape(DIM, tok_per_core, A).transpose(1, 2, 0))
    full = np.concatenate(parts, axis=0)  # (n_tok, A, DIM)
    if full.shape[0] == B * S:
        full = full.reshape(B, S, A, DIM)
    return np.ascontiguousarray(full)


_NC_CACHE = {}


def kernel(x, artery_embed, residual_kv, Wqkv, Wproj, mixer_scale):
    from concourse.bass_utils import run_bass_kernel_spmd

    key = TOK_PER_CORE
    if key not in _NC_CACHE:
        _NC_CACHE[key] = build_program(TOK_PER_CORE)
    nc = _NC_CACHE[key]

    in_maps = host_prep(x, artery_embed, residual_kv, Wqkv, Wproj, mixer_scale)
    res = run_bass_kernel_spmd(nc, in_maps, core_ids=list(range(N_CORES)))
    outs = [r["out_t"] for r in res.results]
    return assemble_output(outs)



# revision 40
# speedup vs baseline: 94.9860x; 1.5520x over previous
"""ArteryMixer Trainium2 kernel: 8-core data-parallel over tokens.

Per-token math (B=2,S=2048,A=8,R=8,DIM=1024,H=8,HD=128,SC=16):
  qkv = concat(x+emb, res) @ Wqkv.T ; q,k rmsnorm ; k_res roped (folded into W);
  scores=elu(q@k.T/sqrt(HD)) ; mixed = scores@v/16 ; out = x + scale*(mixed@Wproj.T)

Device layout strategy (per core, 512 tokens, 8 pipeline blocks of 64):
  - Activations feature-on-partitions (x.T etc., host pre-transposed).
  - QKV/V GEMMs in fp8e4m3 with DoubleRow perf mode (256-deep contraction per
    matmul); weights pre-scaled x32, folded back via Wproj. k_res RoPE folded
    into the weights. Proj GEMM + attention stay bf16.
  - artery-embed biases folded into the GEMM accumulation as rank-8 matmuls
    against a one-hot pattern (no elementwise bias adds).
  - rmsnorm: square (gpsimd) -> ssq per head via ones-column matmul into psum
    rows {0,32,64} (overlapping-M fills keep filler rows finite) -> batched
    ACT sqrt + DVE reciprocal -> rank-1 broadcast matmul + DVE multiply onto
    the slab. rs_q = rsqrt(ssq/128+eps); rs_k = rsqrt(ssq+128eps) folds HD^-.5.
  - attention per 16-token group: scoresT = kT_slice.T @ qT_slice (all cross
    products), elu = (relu(s)-1) + min(exp(s),1) (exp on ACT, rest on DVE),
    block-diag mask*(1/16) on gpsimd kills cross-token terms.
  - mixedT = v.T @ routeT accumulated art+res -> feeds W-stationary proj;
    y.T = projT*mixer_scale + x.T ; host un-transposes the output.
"""

import numpy as np
import ml_dtypes

bf16 = ml_dtypes.bfloat16
f8e4 = ml_dtypes.float8_e4m3
FP8 = True     # qkv/v GEMMs in fp8e4m3 with DoubleRow (2x contraction per MM)
SW = 32.0      # weight pre-scale for fp8 dynamic range; folded out via Wproj

HEADS = 8
HD = 128
DIM = 1024
MD = 1024
A = 8
RKV = 8
SC = 16
EPS = 1.1920929e-07
ROPE_BASE = 10000.0
N_CORES = 8
B, S = 2, 2048
TOK_PER_CORE = (B * S) // N_CORES  # 512
BLK_TOK = 64                        # tokens per pipeline block
NB = TOK_PER_CORE // BLK_TOK        # 8 blocks
CPB = BLK_TOK * 8                   # 512 cols per block (token-major, slot-minor)


def _rope_matrix():
    inv_freq = 1.0 / (ROPE_BASE ** (np.arange(0, HD, 2, dtype=np.float64) / HD))
    c, s = np.cos(inv_freq), np.sin(inv_freq)
    Rm = np.zeros((HD, HD), dtype=np.float64)
    i = np.arange(HD // 2)
    # reference _rope: out1 = x1*c + x2*s ; out2 = -x1*s + x2*c
    Rm[i, i] = c
    Rm[i, i + 64] = s
    Rm[i + 64, i] = -s
    Rm[i + 64, i + 64] = c
    return Rm


def build_program(tok_per_core=TOK_PER_CORE, repeat=1):
    import concourse.bass as bass  # noqa
    import concourse.mybir as mybir
    import concourse.tile as tile
    from concourse import bacc
    from concourse import bass_isa

    dt = mybir.dt
    Alu = mybir.AluOpType
    Act = mybir.ActivationFunctionType

    nb = tok_per_core // BLK_TOK
    COLS = tok_per_core * 8

    nc = bacc.Bacc(None, target_bir_lowering=False)

    gdt = dt.float8e4 if FP8 else dt.bfloat16
    xt_art = nc.dram_tensor("xt_art", [DIM, COLS], dt.bfloat16, kind="ExternalInput")
    if FP8:
        xa8_d = nc.dram_tensor("xa8", [DIM, COLS], gdt, kind="ExternalInput")
        xr8_d = nc.dram_tensor("xr8", [DIM, COLS], gdt, kind="ExternalInput")
    else:
        xt_res = nc.dram_tensor("xt_res", [DIM, COLS], dt.bfloat16,
                                kind="ExternalInput")
    wqkv_t = nc.dram_tensor("wqkv_t", [DIM, 3 * MD], gdt, kind="ExternalInput")
    wv_t = nc.dram_tensor("wv_t", [DIM, MD], gdt, kind="ExternalInput")
    wproj_t = nc.dram_tensor("wproj_t", [MD, DIM], dt.bfloat16, kind="ExternalInput")
    biasqk_d = nc.dram_tensor("biasqk", [8, 2 * MD], dt.bfloat16, kind="ExternalInput")
    biasv_d = nc.dram_tensor("biasv", [8, MD], dt.bfloat16, kind="ExternalInput")
    onehot_d = nc.dram_tensor("onehot", [8, CPB], dt.bfloat16, kind="ExternalInput")
    mask_d = nc.dram_tensor("mask", [128, 128], dt.bfloat16, kind="ExternalInput")
    mscale_d = nc.dram_tensor("mscale", [128, 8], dt.float32, kind="ExternalInput")
    out_t = nc.dram_tensor("out_t", [DIM, COLS], dt.bfloat16, kind="ExternalOutput")

    with tile.TileContext(nc) as tc:
        with (
            tc.tile_pool(name="w", bufs=1) as wpool,
            tc.tile_pool(name="x", bufs=2) as xpool,
            tc.tile_pool(name="slab", bufs=2) as spool,
            tc.tile_pool(name="vslab", bufs=2) as vpool,
            tc.tile_pool(name="mxt", bufs=1) as mxtpool,
            tc.tile_pool(name="nrm", bufs=3) as npool,
            tc.tile_pool(name="att", bufs=3) as fpool,
            tc.tile_pool(name="rtp", bufs=3) as rtpool,
            tc.tile_pool(name="y", bufs=2) as ypool,
            tc.tile_pool(name="mm", bufs=2, space="PSUM") as mmpool,
            tc.tile_pool(name="sc", bufs=1, space="PSUM") as scpool,
            tc.tile_pool(name="mx", bufs=1, space="PSUM") as mxpool,
            tc.tile_pool(name="bc", bufs=1, space="PSUM") as bcpool,
            tc.tile_pool(name="sq", bufs=2, space="PSUM") as sqpool,
        ):
            # ---- resident weights/constants ----
            wqkv_sb = wpool.tile([128, 8, 3 * MD], gdt)
            nc.sync.dma_start(
                wqkv_sb, wqkv_t[:].rearrange("(dc p) f -> p dc f", p=128)
            )
            wv_sb = wpool.tile([128, 8, MD], gdt)
            nc.sync.dma_start(wv_sb, wv_t[:].rearrange("(dc p) f -> p dc f", p=128))
            wproj_sb = wpool.tile([128, 8, DIM], dt.bfloat16)
            nc.sync.dma_start(
                wproj_sb, wproj_t[:].rearrange("(mc p) f -> p mc f", p=128)
            )
            biasqk_sb = wpool.tile([8, 2 * MD], dt.bfloat16)
            nc.sync.dma_start(biasqk_sb, biasqk_d[:])
            biasv_sb = wpool.tile([8, MD], dt.bfloat16)
            nc.sync.dma_start(biasv_sb, biasv_d[:])
            onehot_sb = wpool.tile([8, CPB], dt.bfloat16)
            nc.sync.dma_start(onehot_sb, onehot_d[:])
            mask_sb = wpool.tile([128, 128], dt.bfloat16)
            nc.sync.dma_start(mask_sb, mask_d[:])
            mscale_sb = wpool.tile([128, 8], dt.float32)
            nc.sync.dma_start(mscale_sb, mscale_d[:])
            eps_q = wpool.tile([128, 1], dt.float32)
            nc.vector.memset(eps_q, EPS)
            eps_k = wpool.tile([128, 1], dt.float32)
            nc.vector.memset(eps_k, HD * EPS)
            ones_sb = wpool.tile([128, 128], dt.bfloat16)
            nc.vector.memset(ones_sb, 1.0)

            xa_dram = xt_art[:].rearrange("(dc p) c -> p dc c", p=128)
            if FP8:
                xa8_dram = xa8_d[:].rearrange("(dc p) c -> p dc c", p=128)
                xr8_dram = xr8_d[:].rearrange("(dc p) c -> p dc c", p=128)
            else:
                xr_dram = xt_res[:].rearrange("(dc p) c -> p dc c", p=128)
            yo_dram = out_t[:].rearrange("(dc p) c -> p dc c", p=128)

            def build_gemm_items(blk):
                """Allocate block tiles + return GEMM work-item closures."""
                c0 = blk * CPB
                xa_h = [xpool.tile([128, 4, CPB], dt.bfloat16, tag=f"xa{i}", name=f"xa{i}")
                        for i in range(2)]
                for i in range(2):
                    nc.sync.dma_start(
                        xa_h[i], xa_dram[:, i * 4 : i * 4 + 4, c0 : c0 + CPB]
                    )
                if FP8:
                    xa8_t = xpool.tile([128, 8, CPB], dt.float8e4, tag="xa8")
                    xr8_t = xpool.tile([128, 8, CPB], dt.float8e4, tag="xr8")
                    nc.sync.dma_start(xa8_t, xa8_dram[:, :, c0 : c0 + CPB])
                    nc.sync.dma_start(xr8_t, xr8_dram[:, :, c0 : c0 + CPB])
                    ga, gr = xa8_t, xr8_t
                else:
                    xr_h = [xpool.tile([128, 4, CPB], dt.bfloat16, tag=f"xr{i}",
                                       name=f"xr{i}") for i in range(2)]
                    for i in range(2):
                        nc.sync.dma_start(
                            xr_h[i], xr_dram[:, i * 4 : i * 4 + 4, c0 : c0 + CPB]
                        )
                qT = spool.tile([128, 8, CPB], dt.bfloat16, tag="qT")
                kTa = spool.tile([128, 8, CPB], dt.bfloat16, tag="kTa")
                kTr = spool.tile([128, 8, CPB], dt.bfloat16, tag="kTr")
                va = vpool.tile([128, 4, 8, HD], dt.bfloat16, tag="va")
                vr = vpool.tile([128, 4, 8, HD], dt.bfloat16, tag="vr")
                st = dict(xa_h=xa_h, qT=qT, kTa=kTa, kTr=kTr,
                          va=va, vr=vr, c0=c0)
                slabs = [qT, kTa, kTr]
                items = []
                DR = mybir.MatmulPerfMode.DoubleRow

                def qkv_item(fc):
                    def go():
                        ps = mmpool.tile([128, CPB], dt.float32, tag="mmps")
                        has_bias = fc < 16
                        if FP8:
                            g = gr if fc >= 16 else ga
                            for d2 in range(4):
                                nc.tensor.matmul(
                                    ps,
                                    wqkv_sb[:, 2 * d2 : 2 * d2 + 2,
                                            fc * 128 : (fc + 1) * 128],
                                    g[:, 2 * d2 : 2 * d2 + 2, :],
                                    start=(d2 == 0),
                                    stop=(d2 == 3 and not has_bias),
                                    perf_mode=DR,
                                )
                        else:
                            halves = xr_h if fc >= 16 else xa_h
                            for dc in range(8):
                                nc.tensor.matmul(
                                    ps,
                                    wqkv_sb[:, dc, fc * 128 : (fc + 1) * 128],
                                    halves[dc // 4][:, dc % 4, :],
                                    start=(dc == 0),
                                    stop=(dc == 7 and not has_bias),
                                )
                        if has_bias:
                            # artery-embed bias as a rank-8 matmul appended to
                            # the accumulation group (bias[a,f] x onehot[a,c])
                            nc.tensor.matmul(
                                ps,
                                biasqk_sb[0:8, fc * 128 : (fc + 1) * 128],
                                onehot_sb,
                                start=False, stop=True,
                            )
                        dst = slabs[fc // 8][:, fc % 8, :]
                        nc.scalar.copy(out=dst, in_=ps)
                    return go

                def v_item(isart, rc, vh):
                    def go():
                        dstv = va if isart else vr
                        ps = mmpool.tile([128, 512], dt.float32, tag="mmps")
                        if FP8:
                            g = ga if isart else gr
                            for d2 in range(4):
                                nc.tensor.matmul(
                                    ps,
                                    g[:, 2 * d2 : 2 * d2 + 2,
                                      rc * 128 : (rc + 1) * 128],
                                    wv_sb[:, 2 * d2 : 2 * d2 + 2,
                                          vh * 512 : (vh + 1) * 512],
                                    start=(d2 == 0),
                                    stop=(d2 == 3 and not isart),
                                    perf_mode=DR,
                                )
                        else:
                            halves = xa_h if isart else xr_h
                            for dc in range(8):
                                nc.tensor.matmul(
                                    ps,
                                    halves[dc // 4][:, dc % 4,
                                                    rc * 128 : (rc + 1) * 128],
                                    wv_sb[:, dc, vh * 512 : (vh + 1) * 512],
                                    start=(dc == 0),
                                    stop=(dc == 7 and not isart),
                                )
                        if isart:
                            # rank-8 per-artery bias via the onehot pattern
                            nc.tensor.matmul(
                                ps,
                                onehot_sb[0:8, 0:128],
                                biasv_sb[0:8, vh * 512 : (vh + 1) * 512],
                                start=False, stop=True,
                            )
                        dv = dstv[:, rc, vh * 4 : (vh + 1) * 4, :]
                        nc.scalar.copy(out=dv, in_=ps)
                    return go

                for fc in range(24):
                    items.append(qkv_item(fc))
                for isart in (True, False):
                    for rc in range(4):
                        for vh in range(2):
                            items.append(v_item(isart, rc, vh))
                return st, items

            def build_attn_items(st):
                """Work items for norm + attention + proj of a block."""
                qT, kTa, kTr = st["qT"], st["kTa"], st["kTr"]
                va, vr, xa_h, c0 = st["va"], st["vr"], st["xa_h"], st["c0"]
                items = []

                def norm_items_for(slab, epsv, scv):
                    """rmsnorm a slab: ssq per head via ones-matmul (partition
                    reduce on PE) into rows {0,32,64} of a shared psum bank,
                    batched sqrt+recip on the strided rows, then per-head
                    rank-1 broadcast matmul + DVE multiply."""
                    groups = [(0, [0, 1, 2]), (1, [3, 4, 5]), (2, [6, 7])]
                    rs_tiles = {}
                    for gi, _ in groups:
                        rs_tiles[gi] = npool.tile(
                            [128, CPB], dt.bfloat16, tag="nrs", name=f"nrs{gi}")

                    def stat_item(gi, heads):
                        def go():
                            with nc.allow_low_precision(reason="bf16 norm ~0.4%"):
                                ssq = sqpool.tile([128, CPB], dt.float32, tag="ssq")
                                n = len(heads)
                                np_ = 32 * (n - 1) + 1
                                for j, hh in enumerate(heads):
                                    sq = npool.tile(
                                        [128, CPB], dt.bfloat16, tag="nsq")
                                    nc.gpsimd.tensor_mul(
                                        sq, slab[:, hh, :], slab[:, hh, :])
                                    # fill this head's whole 32-row band so
                                    # the batched sqrt below reads no
                                    # uninitialized rows; only row 32j is
                                    # consumed downstream
                                    m = 32 if j < n - 1 else 1
                                    nc.tensor.matmul(
                                        ssq[32 * j : 32 * j + m, :],
                                        ones_sb[:, 0:m], sq,
                                        start=True, stop=True,
                                    )
                                # one batched sqrt+recip over the contiguous
                                # partition range
                                rt = npool.tile([128, CPB], dt.bfloat16, tag="nrt")
                                nc.scalar.activation(
                                    rt[0:np_, :], ssq[0:np_, :], Act.Sqrt,
                                    bias=epsv[0:np_, :], scale=scv,
                                )
                                nc.vector.reciprocal(
                                    rs_tiles[gi][0:np_, :], rt[0:np_, :])
                        return go

                    def apply_item(gi, j_heads):
                        def go():
                            rs = rs_tiles[gi]
                            for j, hh in j_heads:
                                bc = bcpool.tile([128, CPB], dt.float32, tag="bc")
                                nc.tensor.matmul(
                                    bc,
                                    ones_sb[32 * j : 32 * j + 1, :],
                                    rs[32 * j : 32 * j + 1, :],
                                    start=True, stop=True,
                                )
                                nc.vector.tensor_mul(
                                    slab[:, hh, :], slab[:, hh, :], bc
                                )
                        return go

                    stats = [stat_item(gi, heads) for gi, heads in groups]
                    applies = []
                    for gi, heads in groups:
                        pairs = list(enumerate(heads))
                        applies.append(apply_item(gi, pairs[:2]))
                        if len(pairs) > 2:
                            applies.append(apply_item(gi, pairs[2:]))
                    return stats, applies

                mixedT = mxtpool.tile([128, 8, CPB], dt.bfloat16, tag="mixedT")
                st["mixedT"] = mixedT
                routes = {}

                def scores_item(g, half):
                    def go():
                        gsl = slice(g * 128, (g + 1) * 128)
                        kT = kTa if half == 0 else kTr
                        ps = scpool.tile([128, 8, 128], dt.float32, tag="scps")
                        for h in range(8):
                            nc.tensor.matmul(
                                ps[:, h, :], kT[:, h, gsl], qT[:, h, gsl],
                                start=True, stop=True,
                            )
                        esc = fpool.tile([128, 8, 128], dt.bfloat16, tag="ers")
                        rsc = fpool.tile([128, 8, 128], dt.bfloat16, tag="ers")
                        # elu = (relu(s) - 1) + min(exp(s), 1)
                        nc.scalar.activation(esc, ps, Act.Exp)
                        nc.vector.tensor_scalar(rsc, ps, 0.0, -1.0, Alu.max, Alu.add)
                        nc.vector.scalar_tensor_tensor(
                            out=esc, in0=esc, scalar=1.0, in1=rsc,
                            op0=Alu.min, op1=Alu.add,
                        )
                        route = rtpool.tile([128, 8, 128], dt.bfloat16, tag="rt")
                        nc.gpsimd.tensor_mul(
                            route, esc,
                            mask_sb[:, None, :].to_broadcast((128, 8, 128)),
                        )
                        routes[(g, half)] = route
                    return go

                def mixed_item(g, h0):
                    def go():
                        gsl = slice(g * 128, (g + 1) * 128)
                        mx = mxpool.tile([128, 4, 128], dt.float32, tag="mxps")
                        for i, h in enumerate(range(h0, h0 + 4)):
                            nc.tensor.matmul(
                                mx[:, i, :], va[:, g, h, :],
                                routes[(g, 0)][:, h, :], start=True, stop=False,
                            )
                            nc.tensor.matmul(
                                mx[:, i, :], vr[:, g, h, :],
                                routes[(g, 1)][:, h, :], start=False, stop=True,
                            )
                        nc.scalar.copy(
                            out=mixedT[:, h0 : h0 + 4, gsl], in_=mx)
                    return go

                def proj_item(dc):
                    def go():
                        ps = mmpool.tile([128, CPB], dt.float32, tag="mmps")
                        for h in range(8):
                            nc.tensor.matmul(
                                ps,
                                wproj_sb[:, h, dc * 128 : (dc + 1) * 128],
                                mixedT[:, h, :],
                                start=(h == 0), stop=(h == 7),
                            )
                        yb = ypool.tile([128, CPB], dt.bfloat16, tag="yb")
                        nc.vector.scalar_tensor_tensor(
                            out=yb, in0=ps, scalar=mscale_sb[:, dc : dc + 1],
                            in1=xa_h[dc // 4][:, dc % 4, :],
                            op0=Alu.mult, op1=Alu.add,
                        )
                        nc.sync.dma_start(yo_dram[:, dc, c0 : c0 + CPB], yb)
                    return go

                all_stats, all_applies = [], []
                for slab, epsv, scv in (
                    (qT, eps_q, 1.0 / HD),
                    (kTa, eps_k, 1.0),
                    (kTr, eps_k, 1.0),
                ):
                    stats, applies = norm_items_for(slab, epsv, scv)
                    all_stats.extend(stats)
                    all_applies.extend(applies)
                norm_items = all_stats + all_applies
                return dict(
                    norm=norm_items,
                    groups=[(scores_item(g, 0), scores_item(g, 1),
                             mixed_item(g, 0), mixed_item(g, 4))
                            for g in range(4)],
                    proj=[proj_item(dc) for dc in range(8)],
                )

            def merge(attn, gemm):
                """Structured interleave: norm 3:1 with gemms, then per group
                sc,G,sc,G,G,mx,G, then proj 1:2 with gemms; leftovers last."""
                out = []
                gq = list(gemm)

                def g(n):
                    for _ in range(n):
                        if gq:
                            out.append(gq.pop(0))

                if attn is None:
                    return list(gemm)
                for i, it in enumerate(attn["norm"]):
                    out.append(it)
                    if i % 3 == 2:
                        g(1)
                for sc0, sc1, mxa, mxb in attn["groups"]:
                    out.append(sc0); g(1)
                    out.append(sc1); g(2)
                    out.append(mxa); g(1)
                    out.append(mxb); g(1)
                for p in attn["proj"]:
                    out.append(p); g(2)
                out.extend(gq)
                return out

            blklist = [b for _ in range(repeat) for b in range(nb)]
            prev_st = None
            for i in range(len(blklist) + 1):
                gemm_items = []
                if i < len(blklist):
                    st, gemm_items = build_gemm_items(blklist[i])
                attn = build_attn_items(prev_st) if prev_st is not None else None
                for item in merge(attn, gemm_items):
                    item()
                if i < len(blklist):
                    prev_st = st

    nc.compile()
    return nc


def host_prep(x, artery_embed, residual_kv, Wqkv, Wproj, mixer_scale,
              tok_per_core=TOK_PER_CORE, n_cores=N_CORES):
    T = x.shape[0] * x.shape[1]
    x_flat = np.asarray(x, dtype=np.float32).reshape(T, A, DIM)
    res_flat = np.asarray(residual_kv, dtype=np.float32).reshape(T, RKV, DIM)

    Rm = _rope_matrix()
    sw = SW if FP8 else 1.0
    Wq = np.asarray(Wqkv[0:MD], dtype=np.float64) * sw
    Wk = np.asarray(Wqkv[MD : 2 * MD], dtype=np.float64) * sw
    Wv = np.asarray(Wqkv[2 * MD : 3 * MD], dtype=np.float64) * sw
    Wk_res = np.einsum("de,hec->hdc", Rm, Wk.reshape(HEADS, HD, DIM)).reshape(MD, DIM)

    gnp = f8e4 if FP8 else bf16
    wqkv_t = np.ascontiguousarray(
        np.concatenate([Wq, Wk, Wk_res], axis=0).T
    ).astype(gnp)
    wv_t = np.ascontiguousarray(Wv.T).astype(gnp)
    wproj_t = np.ascontiguousarray(
        np.asarray(Wproj, dtype=np.float64).T / sw
    ).astype(bf16)

    emb = np.asarray(artery_embed, dtype=np.float64)
    bias_q = emb @ Wq.T
    bias_k = emb @ Wk.T
    bias_v = emb @ Wv.T
    bias_cat = np.concatenate([bias_q, bias_k], axis=1)  # (8, 2048)
    biasqk = np.ascontiguousarray(bias_cat).astype(bf16)
    biasv = np.ascontiguousarray(bias_v).astype(bf16)
    onehot = (np.arange(CPB)[None, :] % 8 == np.arange(8)[:, None]).astype(bf16)

    mask = np.zeros((128, 128), dtype=np.float32)
    for t in range(16):
        mask[t * 8 : (t + 1) * 8, t * 8 : (t + 1) * 8] = 1.0 / SC
    mask = mask.astype(bf16)

    mscale = np.ascontiguousarray(
        np.asarray(mixer_scale, dtype=np.float32).reshape(8, 128).T
    )

    shared = dict(
        wqkv_t=wqkv_t, wv_t=wv_t, wproj_t=wproj_t, biasqk=biasqk, biasv=biasv,
        onehot=onehot, mask=mask, mscale=mscale,
    )
    in_maps = []
    for i in range(n_cores):
        sl = slice(i * tok_per_core, (i + 1) * tok_per_core)
        xa_t = np.ascontiguousarray(x_flat[sl].reshape(tok_per_core * A, DIM).T)
        xr_t = np.ascontiguousarray(res_flat[sl].reshape(tok_per_core * RKV, DIM).T)
        m = dict(shared)
        m["xt_art"] = xa_t.astype(bf16)
        if FP8:
            m["xa8"] = xa_t.astype(f8e4)
            m["xr8"] = xr_t.astype(f8e4)
        else:
            m["xt_res"] = xr_t.astype(bf16)
        in_maps.append(m)
    return in_maps


def assemble_output(outs, tok_per_core=TOK_PER_CORE):
    """outs: list of (DIM, tok_per_core*8) bf16 arrays -> (B,S,A,DIM) f32."""
    parts = []
    for o in outs:
        y = np.asarray(o, dtype=np.float32)  # (1024, T*8)
        parts.append(y.reshape(DIM, tok_per_core, A).transpose(1, 2, 0))
    full = np.concatenate(parts, axis=0)  # (n_tok, A, DIM)
    if full.shape[0] == B * S:
        full = full.reshape(B, S, A, DIM)
    return np.ascontiguousarray(full)


_RUNNER_CACHE = {}


def _make_runner():
    """Compile the per-core program once and return a reusable callable.

    Mirrors concourse.bass2jax.run_bass_via_pjrt's multi-core lowering but
    caches the jitted executable across kernel() calls (outputs are fully
    written by the kernel, so the zero initializers are uploaded once and
    not donated).
    """
    import jax
    from jax.experimental.shard_map import shard_map
    from jax.sharding import Mesh, NamedSharding, PartitionSpec

    import concourse.mybir as mybir
    from concourse import bass2jax

    nc = build_program(TOK_PER_CORE)
    bass2jax.install_neuronx_cc_hook()
    partition_name = nc.partition_id_tensor.name if nc.partition_id_tensor else None

    in_names, out_names, out_avals, zero_outs = [], [], [], []
    for alloc in nc.m.functions[0].allocations:
        if not isinstance(alloc, mybir.MemoryLocationSet):
            continue
        name = alloc.memorylocations[0].name
        if alloc.kind == "ExternalInput":
            if name != partition_name:
                in_names.append(name)
        elif alloc.kind == "ExternalOutput":
            out_names.append(name)
            shape = tuple(alloc.tensor_shape)
            dtype = mybir.dt.np(alloc.dtype)
            out_avals.append(jax.core.ShapedArray(shape, dtype))
            zero_outs.append(np.zeros(shape, dtype))

    n_params = len(in_names)
    all_in_names = list(in_names) + list(out_names)
    if partition_name is not None:
        all_in_names.append(partition_name)

    def _body(*args):
        operands = list(args)
        if partition_name is not None:
            operands.append(bass2jax.partition_id_tensor())
        outs = bass2jax._bass_exec_p.bind(
            *operands,
            out_avals=tuple(out_avals),
            in_names=tuple(all_in_names),
            out_names=tuple(out_names),
            lowering_input_output_aliases=(),
            sim_require_finite=True,
            sim_require_nnan=True,
            nc=nc,
        )
        return tuple(outs)

    devices = jax.devices()[:N_CORES]
    mesh = Mesh(np.asarray(devices), ("core",))
    in_specs = (PartitionSpec("core"),) * (n_params + len(out_avals))
    out_specs = (PartitionSpec("core"),) * len(out_names)
    sharded = jax.jit(
        shard_map(_body, mesh=mesh, in_specs=in_specs, out_specs=out_specs,
                  check_rep=False),
        keep_unused=True,
    )
    sharding = NamedSharding(mesh, PartitionSpec("core"))
    state = {}

    def run(in_maps):
        per_core = [[np.asarray(m[name]) for name in in_names] for m in in_maps]
        concat_in = [
            np.concatenate([per_core[c][i] for c in range(N_CORES)], axis=0)
            for i in range(n_params)
        ]
        dev_in = [jax.device_put(a, sharding) for a in concat_in]
        if "zeros" not in state:
            state["zeros"] = [
                jax.device_put(
                    np.zeros((N_CORES * z.shape[0], *z.shape[1:]), z.dtype),
                    sharding)
                for z in zero_outs
            ]
        outs = sharded(*dev_in, *state["zeros"])
        host = [np.asarray(o) for o in outs]
        return [
            {name: host[i].reshape(N_CORES, *out_avals[i].shape)[c]
             for i, name in enumerate(out_names)}
            for c in range(N_CORES)
        ]

    return run


def kernel(x, artery_embed, residual_kv, Wqkv, Wproj, mixer_scale):
    if "run" not in _RUNNER_CACHE:
        _RUNNER_CACHE["run"] = _make_runner()
    run = _RUNNER_CACHE["run"]

    in_maps = host_prep(x, artery_embed, residual_kv, Wqkv, Wproj, mixer_scale)
    out_maps = run(in_maps)
    outs = [m["out_t"] for m in out_maps]
    return assemble_output(outs)

